# revision 25
# baseline (speedup 1.0000x reference)
"""TLGv4 block-sparse self-attention on 8 trn2 NeuronCores.

Sharding: tensor-parallel over the 8 KV groups (1 group = 4 Q heads + 1 K +
1 V head per core). Each core computes its group's QKV projection columns,
RoPE, block-sparse attention for its 4 Q heads, and a row-sharded partial of
the dense output projection. Host sums the 8 partials (+ b_dense).

v2: single fused schedule. The 4 QKV token windows and the 16 attention
pairs are interleaved so the scalar engine (softmax Exp) works during QKV
matmuls and the tensor engine never drains:
  - window n: QKV matmuls (mc-major, 2 PSUM banks), bias via scalar ACT,
    RoPE on DVE, V-transpose via PE (4 transposes into one PSUM bank)
  - pairs 4n..4n+3 right after window n: per chunk, score matmul -> Exp
    (scalar) -> masks (DVE) -> PV matmul one step behind; softmax
    denominators via the ones-column of V; reciprocal on DVE; partition
    broadcast via a K=1 ones matmul into rows 64:128 of the ctx PSUM bank
    (no DRAM bounce); ctx normalize muls on DVE
  - dense matmul units (2 matmuls + copy + DMA per 512-col strip) are
    deferred one pair and spread between chunks as tensor filler
"""
import numpy as np
from contextlib import ExitStack

import concourse.bacc as bacc
import concourse.bass as bass
import concourse.mybir as mybir
import concourse.tile as tile
from concourse.bass_utils import run_bass_kernel_spmd

F32 = mybir.dt.float32
F16 = mybir.dt.float16
AF = mybir.ActivationFunctionType

S = 2048
HID = 2048
D = 64
H_KV = 8
NQ = 4                      # q heads per kv group
GCOLS = (NQ + 2) * D        # 384 qkv columns per group
NPAIR = S // 128            # 16 pairs of 64-token blocks
SCALE = 1.0 / 8.0           # 1/sqrt(D)
ROPE_BASE = 10000.0
N_CORES = 8


def _pair_chunks(i):
    """128-token k-chunks feeding query pair i (blocks 2i, 2i+1)."""
    chunks = list(range(max(0, i - 8), i + 1))
    if i >= 12:
        chunks = [3] + chunks
    return chunks


def _build_nc():
    nc = bacc.Bacc()

    ht = nc.declare_dram_parameter("ht", [HID, S], F16, isOutput=False)
    wq = nc.declare_dram_parameter("wq", [128, 16 * GCOLS], F16, isOutput=False)
    bq = nc.declare_dram_parameter("bq", [128, 3], F32, isOutput=False)
    wd = nc.declare_dram_parameter("wd", [128, 2 * HID], F16, isOutput=False)
    cosq = nc.declare_dram_parameter("cosq", [128, S], F16, isOutput=False)
    sinq = nc.declare_dram_parameter("sinq", [128, S], F16, isOutput=False)
    cosk = nc.declare_dram_parameter("cosk", [64, S], F16, isOutput=False)
    sink = nc.declare_dram_parameter("sink", [64, S], F16, isOutput=False)
    tri = nc.declare_dram_parameter("tri", [128, 512], F16, isOutput=False)
    ident = nc.declare_dram_parameter("ident", [128, 128], F16, isOutput=False)
    out = nc.declare_dram_parameter("out", [S, HID], F32, isOutput=True)

    with tile.TileContext(nc) as tc, ExitStack() as ctx:
        # few pools -> short drain epilogue; rings are per-tag via bufs=
        consts = ctx.enter_context(tc.tile_pool(name="consts", bufs=1))
        hp = ctx.enter_context(tc.tile_pool(name="hp", bufs=32))
        wk = ctx.enter_context(tc.tile_pool(name="wk", bufs=2))
        ps = ctx.enter_context(tc.tile_pool(name="ps", bufs=1, space="PSUM"))
        persist = consts
        rp = att = ob = sm = wk
        psa = pss = psc = psd = ps

        wq_sb = consts.tile([128, 16 * GCOLS], F16)
        wd_sb = consts.tile([128, 2 * HID], F16)
        bq_sb = consts.tile([128, 3], F32)
        cosq_sb = consts.tile([128, S], F16)
        sinq_sb = consts.tile([128, S], F16)
        cosk_sb = consts.tile([64, S], F16)
        sink_sb = consts.tile([64, S], F16)
        tri_sb = consts.tile([128, 512], F16)  # additive causal mask, 4 heads
        id_sb = consts.tile([128, 128], F16)
        expb = consts.tile([128, 1], F32)
        ones_sb = consts.tile([1, 64], F16)
        nc.vector.memset(expb[:], -5.0)
        nc.vector.memset(ones_sb[:], 1.0)

        # Per-queue DMA depth is only ~2 in flight, so each queue delivers
        # transfers serially. Spread the startup-critical stream (wq heads +
        # window-0 hch) across the scalar and sync queues in consumption
        # order; everything needed later (tables, wd) rides gpsimd.
        # wq is host-packed mc-major; the kc0:4 head of each mc slice comes
        # first so window 0's kc-major pass can start immediately.
        for mc in range(3):
            nc.scalar.dma_start(out=wq_sb[:, mc * 2048:mc * 2048 + 512],
                                in_=wq[:, mc * 2048:mc * 2048 + 512])
        nc.gpsimd.dma_start(out=bq_sb[:], in_=bq[:, :])
        nc.gpsimd.dma_start(out=cosq_sb[:], in_=cosq[:, :])
        nc.gpsimd.dma_start(out=sinq_sb[:], in_=sinq[:, :])
        nc.gpsimd.dma_start(out=cosk_sb[:], in_=cosk[:, :])
        nc.gpsimd.dma_start(out=sink_sb[:], in_=sink[:, :])
        nc.gpsimd.dma_start(out=tri_sb[:], in_=tri[:, :])
        nc.gpsimd.dma_start(out=id_sb[:], in_=ident[:, :])
        nc.gpsimd.dma_start(out=wd_sb[:, 0:HID], in_=wd[:, 0:HID])
        nc.gpsimd.dma_start(out=wd_sb[:, HID:2 * HID], in_=wd[:, HID:2 * HID])

        # persistent activations
        qkv = [persist.tile([128, S], F16, tag=f"qkv{m}", name=f"qkv{m}")
               for m in range(3)]
        qS = persist.tile([64, NQ * S], F16)     # [d, pair*512 + h*128 + t]
        kT = persist.tile([64, S], F16)          # [d, t]
        v_sb = persist.tile([128, 16 * 66], F16)  # [t, chunk*66 + (d | ones | pad)]
        ctx_sb = persist.tile([128, 2 * S], F16)  # [(h%2)*64+d, (h//2)*2048 + t]

        v_r = v_sb[:].rearrange("p (c w) -> p c w", w=66)
        nc.vector.memset(v_r[:, :, 64:65], 1.0)
        nc.vector.memset(v_r[:, :, 65:66], 0.0)

        # hidden-state chunks on the sync queue; windows 0-1 up front,
        # windows 2-3 issued as earlier windows start (same queue as the
        # dense output stream, so keep the backlog bounded)
        hch = {}

        def issue_hch(n, kq):
            t = hp.tile([128, 1024], F16, name="hch")
            src = ht[kq * 256:(kq + 1) * 256,
                     n * 512:(n + 1) * 512].rearrange("(c p) t -> p c t", p=128)
            dst = t[:].rearrange("p (c t) -> p c t", c=2)
            # window 0: halves, alternating sync/scalar queues so both
            # deliver at the kc-major consumption cadence
            if n == 0:
                eng = nc.scalar if kq % 2 == 1 else nc.sync
                eng.dma_start(out=dst[:, 0:1], in_=src[:, 0:1])
                eng.dma_start(out=dst[:, 1:2], in_=src[:, 1:2])
            else:
                nc.sync.dma_start(out=dst, in_=src)
            hch[(n, kq)] = t

        # interleave so each queue's FIFO matches need-order; wq tails
        # land on the scalar queue between the early odd chunks
        issue_hch(0, 0)
        issue_hch(0, 1)
        issue_hch(0, 2)
        for mc in range(3):
            nc.scalar.dma_start(out=wq_sb[:, mc * 2048 + 512:(mc + 1) * 2048],
                                in_=wq[:, mc * 2048 + 512:(mc + 1) * 2048])
        for kq in (4, 3, 6, 5, 7):
            issue_hch(0, kq)
        for kq in range(8):
            issue_hch(1, kq)

        # ---- dense backlog (deferred, spread as tensor filler) ----
        backlog = []
        copy_ctr = [0]
        step_ctr = [0]

        def emit_dense_unit():
            j, nn = backlog.pop(0)
            dps = psd.tile([128, 512], F32, tag="d", bufs=1, name="dps")
            nc.tensor.matmul(dps[:], ctx_sb[:, j * 128:(j + 1) * 128],
                             wd_sb[:, nn * 512:(nn + 1) * 512],
                             start=True, stop=False)
            nc.tensor.matmul(dps[:], ctx_sb[:, S + j * 128:S + (j + 1) * 128],
                             wd_sb[:, HID + nn * 512:HID + (nn + 1) * 512],
                             start=False, stop=True)
            ost = ob.tile([128, 512], F32, bufs=4, name="ost")
            if copy_ctr[0] % 3 == 1:
                nc.scalar.copy(ost[:], dps[:])
            else:
                nc.vector.tensor_copy(ost[:], dps[:])
            copy_ctr[0] += 1
            nc.sync.dma_start(
                out=out[j * 128:(j + 1) * 128, nn * 512:(nn + 1) * 512],
                in_=ost[:])

        def filler_step():
            step_ctr[0] += 1
            if backlog and (step_ctr[0] % 2 == 0 or len(backlog) > 6):
                emit_dense_unit()

        # ---- deferred per-pair normalize ----
        # chain for pair j: scalar-copy den row -> SBUF->SBUF scatter DMA to
        # [64,8] (gpsimd queue) -> fast DVE reciprocal -> gather DMA back to
        # [1,512] -> K=1 ones matmul broadcasts into a pss slot (rows 0:64)
        # -> 4 DVE muls normalize ctx. recip/gather run at the next pair's
        # first chunk, bcast+muls at its last chunk, so DMA latency hides.
        pending = {}

        def emit_recip(j):
            st = pending[j]
            r8h = sm.tile([64, 8], F16, tag="r8h", name="r8h")
            with nc.allow_low_precision("fp16 softmax denominators"):
                nc.vector.reciprocal(r8h[:], st["r8"][:])
            rec1 = sm.tile([1, 512], F16, tag="rec1", name="rec1")
            nc.gpsimd.dma_start(
                out=rec1[0:1, :].rearrange("o (p f) -> o p f", p=64),
                in_=r8h[:])
            st["rec1"] = rec1

        def emit_bcast_muls(j):
            st = pending.pop(j)
            ctx_ps = st["ctx"]
            nc.tensor.matmul(ctx_ps[64:128, :], ones_sb[:], st["rec1"][:],
                             start=True, stop=True)
            # DVE reads only one PSUM operand: shift the broadcast to SBUF
            bc = sm.tile([64, 512], F16, tag="bc", name="bc")
            nc.vector.tensor_copy(bc[:], ctx_ps[64:128, :])
            for h in range(NQ):
                nc.vector.tensor_mul(
                    ctx_sb[(h % 2) * 64:(h % 2) * 64 + 64,
                           (h // 2) * S + j * 128:(h // 2) * S + (j + 1) * 128],
                    ctx_ps[0:64, h * 128:(h + 1) * 128],
                    bc[:, h * 128:(h + 1) * 128])
            backlog.extend((j, nn) for nn in range(4))

        def emit_pair(j):
            chunks = _pair_chunks(j)
            n_c = len(chunks)
            ctx_ps = psc.tile([128, 512], F32, tag="c", bufs=2, name="ctx_ps")
            pvq = []
            for idx, c in enumerate(chunks):
                s_ps = pss.tile([128, 512], F32, tag="s", bufs=3, name="s_ps")
                nc.tensor.matmul(s_ps[:], kT[:, c * 128:(c + 1) * 128],
                                 qS[:, j * 512:(j + 1) * 512],
                                 start=True, stop=True)
                if c == j:  # diagonal: additive causal mask on scores
                    nc.vector.tensor_add(s_ps[:], s_ps[:], tri_sb[:])
                ex = att.tile([128, 512], F16, tag="ex", bufs=6, name="ex")
                nc.scalar.activation(ex[:], s_ps[:], AF.Exp, bias=expb[:])
                if j >= 8 and c == j - 8:
                    # first half-block invisible; second half only visible
                    # to the even query block unless it is a vertical block
                    nc.vector.memset(ex[0:64, :], 0.0)
                    if j % 4 != 3:
                        exr = ex[64:128, :].rearrange(
                            "p (hh t) -> p hh t", hh=NQ)
                        nc.vector.memset(exr[:, :, 64:128], 0.0)
                elif j >= 12 and c == 3:
                    # vertical block 7 lives in chunk 3; block 6 invisible
                    nc.vector.memset(ex[0:64, :], 0.0)
                pvq.append((c, ex))
                if idx == 0 and (j - 1) in pending:
                    emit_recip(j - 1)
                elif idx == n_c - 1 and (j - 1) in pending:
                    emit_bcast_muls(j - 1)
                if idx >= 1:
                    c2, ex2 = pvq[idx - 1]
                    nc.tensor.matmul(ctx_ps[0:66, :],
                                     v_sb[:, c2 * 66:(c2 + 1) * 66], ex2[:],
                                     start=(idx == 1), stop=False)
                filler_step()
            c2, ex2 = pvq[n_c - 1]
            nc.tensor.matmul(ctx_ps[0:66, :], v_sb[:, c2 * 66:(c2 + 1) * 66],
                             ex2[:], start=(n_c == 1), stop=True)
            den = sm.tile([1, 512], F32, tag="den", name="den")
            nc.scalar.copy(den[:], ctx_ps[64:65, :])
            r8 = sm.tile([64, 8], F32, tag="r8", name="r8")
            nc.gpsimd.dma_start(
                out=r8[:], in_=den[0:1, :].rearrange("o (p f) -> o p f", p=64))
            pending[j] = {"ctx": ctx_ps, "r8": r8}

        # ---- QKV window: mc-major accumulation, rope hooks between passes ----
        def rope_q(n, ti):
            nsl = slice(n * 512, (n + 1) * 512)
            qt = qkv[ti]
            rot = rp.tile([128, 512], F16, tag="rot", name="rot")
            for blk in range(4):
                src = (blk ^ 1) * 32
                nc.vector.tensor_copy(rot[blk * 32:(blk + 1) * 32, :],
                                      qt[src:src + 32, nsl])
            tmp = rp.tile([128, 512], F16, tag="tmp", name="tmp")
            nc.vector.tensor_mul(tmp[:], qt[:, nsl], cosq_sb[:, nsl])
            nc.vector.tensor_mul(rot[:], rot[:], sinq_sb[:, nsl])
            for half in range(2):  # head 2*ti + half
                h = 2 * ti + half
                dst = qS[:, n * 2048:(n + 1) * 2048].rearrange(
                    "p (pp hh t) -> p pp hh t", hh=NQ, t=128)[:, :, h, :]
                nc.vector.tensor_add(
                    dst,
                    tmp[half * 64:(half + 1) * 64, :].rearrange(
                        "p (pp t) -> p pp t", t=128),
                    rot[half * 64:(half + 1) * 64, :].rearrange(
                        "p (pp t) -> p pp t", t=128))

        def rope_k(n):
            nsl = slice(n * 512, (n + 1) * 512)
            rotk = rp.tile([128, 512], F16, tag="rot", name="rotk")
            nc.vector.tensor_copy(rotk[0:32, :], qkv[2][32:64, nsl])
            nc.vector.tensor_copy(rotk[32:64, :], qkv[2][0:32, nsl])
            tmpk = rp.tile([128, 512], F16, tag="tmp", name="tmpk")
            nc.vector.tensor_mul(tmpk[0:64, :], qkv[2][0:64, nsl],
                                 cosk_sb[:, nsl])
            nc.vector.tensor_mul(rotk[0:64, :], rotk[0:64, :],
                                 sink_sb[:, nsl])
            nc.vector.tensor_add(kT[:, nsl], tmpk[0:64, :], rotk[0:64, :])

        def v_transpose(n):
            pt = psd.tile([128, 512], F16, tag="d", bufs=1, name="pt")
            for cc in range(4):
                c = 4 * n + cc
                nc.tensor.transpose(pt[:, cc * 64:(cc + 1) * 64],
                                    qkv[2][64:128, c * 128:(c + 1) * 128],
                                    id_sb[64:128, 64:128])
            nc.vector.tensor_copy(
                v_r[:, 4 * n:4 * n + 4, 0:64],
                pt[:, 0:256].rearrange("p (c t) -> p c t", t=64))

        def emit_qkv(n):
            nsl = slice(n * 512, (n + 1) * 512)
            if n == 0:
                # kc-major: consume each hch chunk for all 3 output banks
                # the moment it lands (the DMA stream paces this window).
                # Third accumulator borrows a bank from the idle score pool.
                accs = [psa.tile([128, 512], F32, tag="a", bufs=2, name="acc0"),
                        psa.tile([128, 512], F32, tag="a", bufs=2, name="acc1"),
                        pss.tile([128, 512], F32, tag="s", bufs=3, name="acc2")]
                for kc in range(16):
                    for mc in range(3):
                        nc.tensor.matmul(
                            accs[mc][:],
                            wq_sb[:, mc * 2048 + kc * 128:mc * 2048 + (kc + 1) * 128],
                            hch[(n, kc // 2)][:, (kc % 2) * 512:(kc % 2 + 1) * 512],
                            start=(kc == 0), stop=(kc == 15))
                for mc in range(3):
                    nc.scalar.activation(qkv[mc][:, nsl], accs[mc][:],
                                         AF.Identity, bias=bq_sb[:, mc:mc + 1])
                    if mc < 2:
                        rope_q(n, mc)
                    else:
                        rope_k(n)
                        v_transpose(n)
                return
            for mc in range(3):
                acc = psa.tile([128, 512], F32, tag="a", bufs=2, name="acc")
                for kc in range(16):
                    nc.tensor.matmul(
                        acc[:],
                        wq_sb[:, mc * 2048 + kc * 128:mc * 2048 + (kc + 1) * 128],
                        hch[(n, kc // 2)][:, (kc % 2) * 512:(kc % 2 + 1) * 512],
                        start=(kc == 0), stop=(kc == 15))
                nc.scalar.activation(qkv[mc][:, nsl], acc[:], AF.Identity,
                                     bias=bq_sb[:, mc:mc + 1])
                if mc < 2:
                    rope_q(n, mc)
                else:
                    rope_k(n)
                    v_transpose(n)
                if backlog:
                    emit_dense_unit()

        for n in range(4):
            if n in (1, 2):
                for kq in range(8):
                    issue_hch(n + 1, kq)
            emit_qkv(n)
            for i in range(4 * n, 4 * n + 4):
                emit_pair(i)

        # tail: pair 15's normalize + remaining dense strips; dense units
        # between the recip chain's DMA hops hide their latency
        emit_recip(15)
        for _ in range(min(2, len(backlog))):
            emit_dense_unit()
        emit_bcast_muls(15)
        while backlog:
            emit_dense_unit()

    nc.finalize()
    return nc


_NC_CACHE = {}


def _get_nc():
    if "nc" not in _NC_CACHE:
        _NC_CACHE["nc"] = _build_nc()
    return _NC_CACHE["nc"]


def _host_inputs(hidden_states, w_qkv, b_qkv, w_dense):
    h = np.asarray(hidden_states, dtype=np.float32).reshape(S, HID)
    w_qkv = np.asarray(w_qkv, dtype=np.float32)
    b_qkv = np.asarray(b_qkv, dtype=np.float32)
    w_dense = np.asarray(w_dense, dtype=np.float32)

    ht = np.ascontiguousarray(h.T).astype(np.float16)

    inv = 1.0 / (ROPE_BASE ** (np.arange(0, D, 2, dtype=np.float32) / D))
    ang = np.arange(S, dtype=np.float32)[:, None] * inv[None, :]   # [S, 32]
    cosT = np.ascontiguousarray(np.cos(ang).T.astype(np.float32))  # [32, S]
    sinT = np.ascontiguousarray(np.sin(ang).T.astype(np.float32))
    cosq = (np.tile(cosT, (4, 1)) * SCALE).astype(np.float16)
    sinq = (np.concatenate([-sinT, sinT, -sinT, sinT], 0) * SCALE).astype(np.float16)
    cosk = np.tile(cosT, (2, 1)).astype(np.float16)
    sink = np.concatenate([-sinT, sinT], 0).astype(np.float16)

    trineg = np.where(np.triu(np.ones((128, 128), np.bool_)), 0.0, -30.0)
    tri = np.tile(trineg.astype(np.float16), (1, 4))   # [128, 512]
    ident = np.eye(128, dtype=np.float16)

    in_maps = []
    for g in range(N_CORES):
        wqg = w_qkv[g * GCOLS:(g + 1) * GCOLS, :].T          # [HID, 384]
        # mc-major packing: [p, mc*2048 + kc*128 + g_col]
        wq_t = np.ascontiguousarray(
            wqg.reshape(16, 128, 3, 128).transpose(1, 2, 0, 3).reshape(
                128, 16 * GCOLS)).astype(np.float16)
        bqg = np.ascontiguousarray(
            b_qkv[g * GCOLS:(g + 1) * GCOLS].reshape(3, 128).T)
        wdg = w_dense[:, g * NQ * D:(g + 1) * NQ * D].T      # [256, HID]
        wd_t = np.ascontiguousarray(
            wdg.reshape(2, 128, HID).transpose(1, 0, 2).reshape(128, 2 * HID)).astype(np.float16)
        in_maps.append({
            "ht": ht, "wq": wq_t, "bq": bqg, "wd": wd_t,
            "cosq": np.ascontiguousarray(cosq), "sinq": np.ascontiguousarray(sinq),
            "cosk": np.ascontiguousarray(cosk), "sink": np.ascontiguousarray(sink),
            "tri": tri, "ident": ident,
        })
    return in_maps


def run_device(hidden_states, w_qkv, b_qkv, w_dense, **run_kwargs):
    nc = _get_nc()
    in_maps = _host_inputs(hidden_states, w_qkv, b_qkv, w_dense)
    return run_bass_kernel_spmd(nc, in_maps, list(range(N_CORES)), **run_kwargs)


def kernel(hidden_states, w_qkv, b_qkv, w_dense, b_dense):
    res = run_device(hidden_states, w_qkv, b_qkv, w_dense)
    acc = np.zeros((S, HID), dtype=np.float32)
    for r in res.results:
        acc += r["out"]
    acc += np.asarray(b_dense, dtype=np.float32)[None, :]
    return acc.reshape(1, S, HID)
